# revision 31
# baseline (speedup 1.0000x reference)
"""Trainium2 Bass kernel for nn_AttentionBlock (B=4, H=W=64, C=256, D=32).

Sharding: 8 shards = 4 samples x 2 query-halves. Each core gets the full
sample (rows reordered so its 2048 query rows come first), computes K and
the fused V@Wo projection for all 4096 keys, and attention + residual for
its 2048 queries. No collectives needed.

Over the previous 98.1us version (measured ~90.6us):
  - Phase B (q/k and W=x@(wv@wo) projections) runs fp8-e4m3 DoubleRow: x
    arrives as one f8 [128,2,4096] tensor (half the DMA), each projection
    is a single DR matmul per chunk (half the PE time).
  - DMA issue order is latency-aware: SDMA engines round-robin across all
    in-flight transfers, so the first two x chunks go as small early
    DMAs, later pairs and the 2MB residual load are issued inside the
    loop; outputs are stored as 8 paired DMAs. (Sync issue cost is
    ~650ns per DMA and a transfer only completes at aggregate-BW share.)
  - A 30-matmul junk warmup bridges the DMA wait so the PE stays busy
    through the HAM activity window and the clock gate opens (1.2 ->
    2.4 GHz) at ~10.6us and stays open, instead of oscillating.
  - Scores run 2-way row-tiled (64-deep contraction = 2 replicated D=32
    bands): tiles (0,0)/(64,0) write different PSUM banks concurrently,
    roughly halving score matmul occupancy; emitted in 3-superstep
    batches (matching the 3 PSUM score slots) to bound tile-mode-switch
    drains.
  - The softmax exp stream is split: 54 supersteps use the ScalarE Exp
    LUT (~1.0us per [128,1024] PSUM tile); 10 are offloaded to VectorE
    via a Schraudolph bit-trick exp writing f8e4m3 bits directly
    (bits8 = 8*log2e*(s-2) + 56, via the HW's saturating
    round-to-nearest f32->uint8 convert), accuracy-neutral next to the
    f8 quantization both exp paths share. ScalarE also absorbs the first
    four W evacuations during its ramp stalls.
  - Epilogue: accumulators are copied out of PSUM immediately (freeing
    the two pa banks for the next group's attend), and the divide +
    residual run one superstep later as a single scalar_tensor_tensor.
  - LAG 16 -> 8: the attend stream trails closer, cutting the drain tail.
"""

import sys

if "/opt/trn_rl_repo" not in sys.path:
    sys.path.insert(0, "/opt/trn_rl_repo")

import numpy as np
import ml_dtypes

BF16 = ml_dtypes.bfloat16
F8 = ml_dtypes.float8_e4m3

# Problem constants
B, HH, WW, C = 4, 64, 64, 256
D = 32
N = HH * WW          # 4096 keys per sample
NQ = N // 2          # 2048 queries per core
NCORES = 8

LAG = 8              # attend trails the scores/exp stream by LAG supersteps
WARMUP_MMS = 30
# Supersteps whose exp runs on VectorE (Schraudolph) instead of ScalarE;
# spaced >=4 apart, away from epilogue supersteps (vv = 8k+7+LAG).
DVE_EXP = frozenset({18, 22, 26, 34, 38, 42, 50, 54, 58, 62})
SCH_A = float(8.0 * np.log2(np.e))             # 11.5416: f8e4m3 bits/ln
SCH_B = float(56.0 - 2.0 * 8.0 * np.log2(np.e))  # f8 bias, shift=2 folded
# phase-B emission: vv -> (qk chunks, w_pass chunks)
PHASE_B = {
    0: ((0, 1), (0,)),
    2: ((2, 3), (1,)),
    4: ((4, 5), (2,)),
    5: ((), (3,)),
    6: ((6, 7), (4,)),
    7: ((), (5,)),
    8: ((), (6,)),
    9: ((), (7,)),
}

_compiled_cache = {}


def _build(use_bias: bool):
    from contextlib import ExitStack
    from concourse import bacc, tile, mybir

    f32 = mybir.dt.float32
    bf = mybir.dt.bfloat16
    f8 = mybir.dt.float8e4
    u8 = mybir.dt.uint8
    DR = mybir.MatmulPerfMode.DoubleRow

    nc = bacc.Bacc("TRN2", target_bir_lowering=False, debug=False, num_devices=NCORES)

    if use_bias:
        xT_d = nc.dram_tensor("xT", [128, 2, N], bf, kind="ExternalInput")
        wb_d = nc.dram_tensor("wblob", [128, 1024], bf, kind="ExternalInput")
        wbias_d = nc.dram_tensor("wbias", [1, 512], bf, kind="ExternalInput")
    else:
        xT8_d = nc.dram_tensor("xT8", [128, 2, N], f8, kind="ExternalInput")
        wb8_d = nc.dram_tensor("wblob8", [128, 2, 512], f8, kind="ExternalInput")
    # residual, pre-transposed on host to [128, 16, 256]
    xq32_d = nc.dram_tensor("xq32", [128, 16, C], f32, kind="ExternalInput")
    # output, host un-permutes [8, 128, 2, 256] -> [2048, 256]
    out_d = nc.dram_tensor("out", [8, 128, 2, C], f32, kind="ExternalOutput")

    Exp = mybir.ActivationFunctionType.Exp
    # undo 2x band replication (64-deep tiled score contraction) + 1/sqrt(D)
    SC2 = float(1.0 / (2.0 * np.sqrt(np.float32(D))))
    Add = mybir.AluOpType.add
    Mult = mybir.AluOpType.mult

    with tile.TileContext(nc) as tc:
        with ExitStack() as ctx:
            const = ctx.enter_context(tc.tile_pool(name="const", bufs=1))
            big = ctx.enter_context(tc.tile_pool(name="big", bufs=1))
            xbp = ctx.enter_context(tc.tile_pool(name="xbp", bufs=4))
            expp = ctx.enter_context(tc.tile_pool(name="expp", bufs=16))
            small = ctx.enter_context(tc.tile_pool(name="small", bufs=2))
            # PSUM: 3 x [128,1024] working tiles (6 banks; shared by phase-B
            # projections and score matmuls) + 2 pa accumulator banks = 8.
            ps_sc = ctx.enter_context(tc.tile_pool(name="ps_sc", bufs=3, space="PSUM"))
            ps_pa = ctx.enter_context(tc.tile_pool(name="ps_pa", bufs=2, space="PSUM"))

            # ---- PE warm-up on junk data while the first DMAs land ----
            wu = const.tile([128, 128], bf, tag="wu")
            nc.gpsimd.memset(wu[:], 1.0)
            pwarm = ps_sc.tile([128, 1024], f32, tag="sc", name="pwarm")
            for _ in range(WARMUP_MMS):
                nc.tensor.matmul(pwarm[:, 0:128], wu[:], wu[:], start=True, stop=True)

            # ---- weights (one blob DMA) ----
            if use_bias:
                wall = const.tile([128, 1024], bf, tag="wall")
                nc.sync.dma_start(out=wall[:], in_=wb_d[:])
                wq0 = wall[:, 0:128]
                wq1 = wall[:, 128:256]
                wk0 = wall[:, 256:384]
                wk1 = wall[:, 384:512]
                wvo0 = wall[:, 512:768]
                wvo1 = wall[:, 768:1024]
                ones_row = const.tile([1, 512], bf, tag="ones_row")
                nc.gpsimd.memset(ones_row[:], 1.0)
                wbias = const.tile([1, 512], bf, tag="wbias")
                nc.sync.dma_start(out=wbias[:], in_=wbias_d[:])
                wqb = wbias[:, 0:128]
                wkb = wbias[:, 128:256]
                wvob = wbias[:, 256:512]
            else:
                w8all = const.tile([128, 2, 512], f8, tag="w8all")
                nc.sync.dma_start(out=w8all[:], in_=wb8_d[:])

            # ---- phase B inputs ----
            # SDMA engines round-robin across ALL in-flight transfers, so
            # the issue order IS the priority order: chunks 0 and 1 go as
            # small single DMAs (low latency), later pairs and the big
            # residual load are issued inside the loop, after the data they
            # would otherwise starve.
            xT_src = xT_d if use_bias else xT8_d
            xdt = bf if use_bias else f8
            xbs = []
            for s in range(2):
                xp = xbp.tile([128, 2, 512], xdt, tag="xbs", bufs=2)
                nc.sync.dma_start(out=xp[:], in_=xT_src[:, :, 512 * s : 512 * s + 512])
                xbs.append(xp[:])
            xpairs = []
            for i in range(1, 4):
                xp = xbp.tile([128, 2, 1024], xdt, tag="xb", bufs=3)
                xpairs.append((xp, i))
                xbs.append(xp[:, :, 0:512])
                xbs.append(xp[:, :, 512:1024])

            # residual x for the epilogue: one DMA, resident in SBUF
            xqall = big.tile([128, 16, C], f32, tag="xqall")

            # Persistent SBUF: qT replicated across the 4 partition bands of
            # 32 (read as 2 bands of 64 by the tiled score matmuls), kT in 2
            # bands of 64 (band 0: chunks 4s/4s+1, band 1: chunks 4s+2/4s+3),
            # and W (= V@Wo) rows with a ones column at 256 for the softmax
            # denominator.
            qT4 = big.tile([128, NQ], bf, tag="qT4")
            kT4 = big.tile([128, N], bf, tag="kT4")
            wsb = big.tile([128, 16, 2, 272], f8, tag="wsb")
            nc.vector.memset(wsb[:, :, :, 256:257], 1.0)
            expbias = const.tile([128, 1], f32, tag="expbias")
            nc.vector.memset(expbias[:], -2.0)

            def qk_chunk(s):
                # q (chunks 0-3) and k share one [128,1024] psum tile
                p = ps_sc.tile([128, 1024], f32, tag="sc", name=f"pqk{s}")
                if use_bias:
                    if s < 4:
                        nc.tensor.matmul(p[:, 0:512], wq0, xbs[s][:, 0, :], start=True, stop=False)
                        nc.tensor.matmul(p[:, 0:512], wq1, xbs[s][:, 1, :], start=False, stop=False)
                        nc.tensor.matmul(p[:, 0:512], wqb, ones_row[:], start=False, stop=True)
                    nc.tensor.matmul(p[:, 512:1024], wk0, xbs[s][:, 0, :], start=True, stop=False)
                    nc.tensor.matmul(p[:, 512:1024], wk1, xbs[s][:, 1, :], start=False, stop=False)
                    nc.tensor.matmul(p[:, 512:1024], wkb, ones_row[:], start=False, stop=True)
                else:
                    if s < 4:
                        nc.tensor.matmul(p[:, 0:512], w8all[:, :, 0:128], xbs[s], start=True, stop=True, perf_mode=DR)
                    nc.tensor.matmul(p[:, 512:1024], w8all[:, :, 128:256], xbs[s], start=True, stop=True, perf_mode=DR)
                # k carries the softmax scale and the 1/2 band-contraction
                # factor; band 0 <- chunks 4s,4s+1 ; band 1 <- 4s+2,4s+3
                # (psum k bands are 4 identical replicas, read diagonally).
                # k first: it gates the score matmuls.
                nc.vector.tensor_scalar(
                    kT4[0:64, 256 * s : 256 * s + 256], p[0:64, 512:768], SC2, None, Mult
                )
                nc.vector.tensor_scalar(
                    kT4[64:128, 256 * s : 256 * s + 256], p[64:128, 768:1024], SC2, None, Mult
                )
                if s < 4:
                    nc.vector.tensor_copy(qT4[:, 512 * s : 512 * s + 512], p[:, 0:512])

            def w_pass(s):
                # W = x @ (wv@wo): 4 key chunks of 128 per xb in one psum tile
                pw = ps_sc.tile([128, 1024], f32, tag="sc", name=f"pw{s}")
                for j2 in range(4):
                    off = 128 * j2
                    if use_bias:
                        nc.tensor.matmul(pw[:, 256 * j2 : 256 * j2 + 256], xbs[s][:, 0, off : off + 128], wvo0, start=True, stop=False)
                        nc.tensor.matmul(pw[:, 256 * j2 : 256 * j2 + 256], xbs[s][:, 1, off : off + 128], wvo1, start=False, stop=False)
                        nc.tensor.matmul(pw[:, 256 * j2 : 256 * j2 + 256], ones_row[:, 0:128], wvob, start=False, stop=True)
                    else:
                        nc.tensor.matmul(pw[:, 256 * j2 : 256 * j2 + 256], xbs[s][:, :, off : off + 128], w8all[:, :, 256:512], start=True, stop=True, perf_mode=DR)
                # early W evacuations ride the ScalarE (its ramp exps are
                # chain-stalled anyway, and they were clogging the DVE queue
                # ahead of the k evacuations); later ones stay on VectorE.
                if s < 4 and not use_bias:
                    nc.scalar.copy(wsb[:, 2 * s : 2 * s + 2, :, 0:256], pw[:])
                else:
                    nc.vector.tensor_copy(wsb[:, 2 * s : 2 * s + 2, :, 0:256], pw[:])

            def scores_mms(g, u):
                # 2-way row tiling: T0 (sbuf partitions 0-63) does chunks
                # 4u,4u+1 into bank A; T8 (64-127) does 4u+2,4u+3 into bank
                # B. Interleaved emission for cross-tile concurrency.
                pst = ps_sc.tile([128, 1024], f32, tag="sc", name=f"ps{g}_{u}")
                qs = slice(256 * g, 256 * g + 256)
                k0 = 256 * u
                nc.tensor.matmul(pst[:, 0:256], kT4[0:64, k0 : k0 + 128], qT4[0:64, qs], start=True, stop=True, tile_position=(0, 0))
                nc.tensor.matmul(pst[:, 512:768], kT4[64:128, k0 : k0 + 128], qT4[64:128, qs], start=True, stop=True, tile_position=(64, 0))
                nc.tensor.matmul(pst[:, 256:512], kT4[0:64, k0 + 128 : k0 + 256], qT4[0:64, qs], start=True, stop=True, tile_position=(0, 0))
                nc.tensor.matmul(pst[:, 768:1024], kT4[64:128, k0 + 128 : k0 + 256], qT4[64:128, qs], start=True, stop=True, tile_position=(64, 0))
                return pst

            def epilogue_evac(g, pa_a, pa_b):
                # Evacuate the accumulators fast so the next group's attend
                # matmuls can reuse the two pa PSUM banks; the divide +
                # residual happen off the critical path a superstep later.
                pasb = small.tile([128, 2, 257], f32, tag="pasb", bufs=2)
                nc.vector.tensor_copy(pasb[:, 0, :], pa_a[:, 0:257])
                nc.vector.tensor_copy(pasb[:, 1, :], pa_b[:, 0:257])
                return pasb

            def epilogue_tail(g, pasb):
                ot = small.tile([128, 2, C], f32, tag="ot", bufs=3)
                for h in range(2):
                    qb = 2 * g + h
                    rec = small.tile([128, 1], f32, tag="rec")
                    nc.vector.reciprocal(rec[:], pasb[:, h, 256:257])
                    # out = attended/denom + x in one DVE op
                    nc.vector.scalar_tensor_tensor(
                        ot[:, h, :], pasb[:, h, 0:256], rec[:], xqall[:, qb, :], Mult, Add
                    )
                nc.sync.dma_start(out=out_d[g], in_=ot[:])

            # ---- software pipeline ----
            # scores/exp stream order: chunk-diagonal over groups 0-1 during
            # the ramp (both only need q chunk 0 + k chunk u), then
            # group-major. Scores are emitted in PAIRS of supersteps (even
            # vv) so the PE switches tile mode at most twice per pair.
            pa_tiles = {}
            ets = {}
            pending_scores = {}
            pending_epi = []
            NSS = 8 * 8
            sched_scores = [(g, u) for u in range(8) for g in (0, 1)] + [
                (g, u) for g in range(2, 8) for u in range(8)
            ]
            sched_attend = [(g, u) for g in range(8) for u in range(8)]

            def do_exp(vv):
                g, u = sched_scores[vv]
                pst = pending_scores.pop(vv)
                et = expp.tile([128, 2, 2, 256], f8, tag="e")
                if vv in DVE_EXP and not use_bias:
                    # Schraudolph exp on VectorE, straight to f8 bits:
                    # bits8 = A*s + B (the -2 shift folded into B); the HW
                    # f32->uint8 convert rounds-to-nearest-even and
                    # SATURATES [0,255], so score underflow lands exactly on
                    # f8 zero. The approximation error is below the f8
                    # quantization both exp paths share.
                    nc.vector.tensor_scalar(et[:].bitcast(u8), pst[:], SCH_A, SCH_B, Mult, Add)
                else:
                    # exp(s - 2): constant shift keeps exp within fp8-e4m3
                    # range (max score ~7 -> e^5 = 148 < 240); the
                    # ones-column denominator sees the same shift, so the
                    # normalized ratio is exact. The first two supersteps
                    # run as two half-tile ACTIVATEs so the exp stream
                    # starts as soon as the T0 score pair (PSUM bank A) is
                    # done, without waiting for the T8 pair.
                    if vv < 2:
                        nc.scalar.activation(et[:, 0, :, :], pst[:, 0:512], Exp, bias=expbias[:])
                        nc.scalar.activation(et[:, 1, :, :], pst[:, 512:1024], Exp, bias=expbias[:])
                    else:
                        nc.scalar.activation(et[:], pst[:], Exp, bias=expbias[:])
                ets[(g, u)] = et

            for vv in range(NSS + LAG):
                if vv == 0:
                    xp, i = xpairs[0]
                    nc.sync.dma_start(out=xp[:], in_=xT_src[:, :, 1024 * i : 1024 * i + 1024])
                elif vv == 2:
                    xp, i = xpairs[1]
                    nc.sync.dma_start(out=xp[:], in_=xT_src[:, :, 1024 * i : 1024 * i + 1024])
                elif vv == 4:
                    xp, i = xpairs[2]
                    nc.sync.dma_start(out=xp[:], in_=xT_src[:, :, 1024 * i : 1024 * i + 1024])
                elif vv == 6:
                    nc.sync.dma_start(out=xqall[:], in_=xq32_d[:])
                qks, ws = PHASE_B.get(vv, ((), ()))
                while pending_epi and pending_epi[0][0] != (vv - 1 - LAG) // 8:
                    g_e, pasb_e = pending_epi.pop(0)
                    epilogue_tail(g_e, pasb_e)
                for s in qks:
                    qk_chunk(s)
                for s in ws:
                    w_pass(s)
                if vv < NSS:
                    if vv % 3 == 0:
                        for v2 in range(vv, min(vv + 3, NSS)):
                            pending_scores[v2] = scores_mms(*sched_scores[v2])
                    do_exp(vv)
                va = vv - LAG
                if 0 <= va < NSS:
                    g_p, u_p = sched_attend[va]
                    if u_p == 0:
                        pa_tiles[2 * g_p] = ps_pa.tile([128, 512], f32, tag="pa", name=f"pa{2 * g_p}")
                        pa_tiles[2 * g_p + 1] = ps_pa.tile([128, 512], f32, tag="pa", name=f"pa{2 * g_p + 1}")
                    et_p = ets.pop((g_p, u_p))
                    for jp in range(2):
                        P = 2 * u_p + jp  # wsb pair: key chunks 2P, 2P+1
                        for h in range(2):
                            nc.tensor.matmul(
                                pa_tiles[2 * g_p + h][:, 0:257],
                                et_p[:, jp, :, 128 * h : 128 * h + 128],
                                wsb[:, P, :, 0:257],
                                start=(P == 0),
                                stop=(P == 15),
                                perf_mode=DR,
                            )
                    if u_p == 7:
                        if g_p == 7:
                            # final group: nothing reuses the pa banks, so
                            # skip the evacuation hop and write out directly
                            ot7 = small.tile([128, 2, C], f32, tag="ot", bufs=3)
                            for h in range(2):
                                rec7 = small.tile([128, 1], f32, tag="rec")
                                nc.vector.reciprocal(rec7[:], pa_tiles[2 * g_p + h][:, 256:257])
                                nc.vector.scalar_tensor_tensor(
                                    ot7[:, h, :], pa_tiles[2 * g_p + h][:, 0:256], rec7[:],
                                    xqall[:, 2 * g_p + h, :], Mult, Add,
                                )
                            nc.sync.dma_start(out=out_d[g_p], in_=ot7[:])
                        else:
                            pending_epi.append((g_p, epilogue_evac(g_p, pa_tiles[2 * g_p], pa_tiles[2 * g_p + 1])))
                        del pa_tiles[2 * g_p]
                        del pa_tiles[2 * g_p + 1]
            for g_e, pasb_e in pending_epi:
                epilogue_tail(g_e, pasb_e)

    nc.compile()
    return nc


def _get_compiled(use_bias: bool):
    key = bool(use_bias)
    if key not in _compiled_cache:
        _compiled_cache[key] = _build(use_bias)
    return _compiled_cache[key]


def _prep(x, wq, bq, wk, bk, wv, bv, wo, bo):
    xf = np.ascontiguousarray(np.asarray(x, dtype=np.float32)).reshape(B, N, C)
    wq = np.asarray(wq, np.float32)
    bq = np.asarray(bq, np.float32)
    wk = np.asarray(wk, np.float32)
    bk = np.asarray(bk, np.float32)
    wv = np.asarray(wv, np.float32)
    bv = np.asarray(bv, np.float32)
    wo = np.asarray(wo, np.float32)
    bo = np.asarray(bo, np.float32)

    use_bias = not (
        np.all(bq == 0) and np.all(bk == 0) and np.all(bv == 0) and np.all(bo == 0)
    )

    # Weights go in UNSCALED; the softmax scale and the band-contraction
    # factor are applied at the k evacuation.
    wvo = (wv @ wo).astype(BF16)  # fold wo into the value projection
    if use_bias:
        wq_rep = np.tile(wq, (1, 4)).astype(BF16)  # [256, 128]
        wk_rep = np.tile(wk, (1, 4)).astype(BF16)
        wblob = np.ascontiguousarray(
            np.concatenate(
                [wq_rep[0:128], wq_rep[128:256], wk_rep[0:128], wk_rep[128:256],
                 wvo[0:128], wvo[128:256]],
                axis=1,
            )
        )  # [128, 1024]
        wbias = np.ascontiguousarray(
            np.concatenate(
                [np.tile(bq, 4), np.tile(bk, 4), bv @ wo], 0
            )[None, :]
        ).astype(BF16)  # [1, 512]
    else:
        # fp8 DoubleRow packing: [ki, ko, col] = w[ki + 128*ko, col]
        wq_rep = np.tile(wq, (1, 4)).astype(np.float32)
        wk_rep = np.tile(wk, (1, 4)).astype(np.float32)
        blob = np.concatenate([wq_rep, wk_rep, wvo.astype(np.float32)], axis=1)  # [256, 512]
        wblob8 = np.ascontiguousarray(
            blob.reshape(2, 128, 512).transpose(1, 0, 2).astype(F8)
        )  # [128, 2, 512]

    in_maps = []
    for core in range(NCORES):
        b, h = divmod(core, 2)
        if h == 0:
            xo = xf[b]
        else:
            xo = np.concatenate([xf[b, NQ:], xf[b, :NQ]], 0)
        # channel-major transpose on host: [256, 4096] -> [128, 2, 4096]
        xT = np.ascontiguousarray(
            xo.T.reshape(2, 128, N).transpose(1, 0, 2)
        )
        xq = np.ascontiguousarray(xo[:NQ])
        if use_bias:
            xq = xq + bo[None, :]
        # residual pre-transposed to [128 partitions, 16 chunks, 256]
        xqt = np.ascontiguousarray(xq.reshape(16, 128, C).transpose(1, 0, 2))
        if use_bias:
            im = {"xT": xT.astype(BF16), "xq32": xqt, "wblob": wblob, "wbias": wbias}
        else:
            im = {"xT8": xT.astype(F8), "xq32": xqt, "wblob8": wblob8}
        in_maps.append(im)
    return in_maps, use_bias


def _gather(results):
    out = np.empty((B, N, C), np.float32)
    for core in range(NCORES):
        b, h = divmod(core, 2)
        # device layout [8 groups, 128, 2, 256] -> [2048, 256]
        o = results[core]["out"].reshape(8, 128, 2, C).transpose(0, 2, 1, 3).reshape(NQ, C)
        out[b, NQ * h : NQ * (h + 1)] = o
    return out.reshape(B, HH, WW, C)


def kernel(x, wq, bq, wk, bk, wv, bv, wo, bo):
    from concourse.bass_utils import run_bass_kernel_spmd

    in_maps, use_bias = _prep(x, wq, bq, wk, bk, wv, bv, wo, bo)
    nc = _get_compiled(use_bias)
    res = run_bass_kernel_spmd(nc, in_maps, core_ids=list(range(NCORES)))
    return _gather(res.results)


def _ensure_ntff_hook():
    """The agent image's antenv stub lacks axon_hooks; synthesize it so
    run_bass_kernel_spmd(trace=True) can NTFF-profile via libaxon_pjrt."""
    import types

    try:
        from antenv.axon_hooks import get_axon_ntff_profile_hook  # noqa: F401
        return
    except ImportError:
        pass
    import antenv
    from trn_agent_boot.trn_boot import _ntff_profile_via_ctypes

    mod = types.ModuleType("antenv.axon_hooks")
    state = {"h": _ntff_profile_via_ctypes("/opt/axon/libaxon_pjrt.so")}
    mod.get_axon_ntff_profile_hook = lambda: state["h"]
    mod.set_axon_ntff_profile_hook = lambda h: state.__setitem__("h", h)
    sys.modules["antenv.axon_hooks"] = mod
    antenv.axon_hooks = mod


def run_traced(inputs, **kw):
    """For test.py: run with NTFF profiling; returns (output, BassKernelResults)."""
    from concourse.bass_utils import run_bass_kernel_spmd

    _ensure_ntff_hook()

    in_maps, use_bias = _prep(**inputs)
    nc = _get_compiled(use_bias)
    res = run_bass_kernel_spmd(nc, in_maps, core_ids=list(range(NCORES)), trace=True, **kw)
    return _gather(res.results), res


# revision 32
# speedup vs baseline: 1.0034x; 1.0034x over previous
"""Trainium2 Bass kernel for nn_AttentionBlock (B=4, H=W=64, C=256, D=32).

Sharding: 8 shards = 4 samples x 2 query-halves. Each core gets the full
sample (rows reordered so its 2048 query rows come first), computes K and
the fused V@Wo projection for all 4096 keys, and attention + residual for
its 2048 queries. No collectives needed.

Over the previous 98.1us version (measured ~90.6us):
  - Phase B (q/k and W=x@(wv@wo) projections) runs fp8-e4m3 DoubleRow: x
    arrives as one f8 [128,2,4096] tensor (half the DMA), each projection
    is a single DR matmul per chunk (half the PE time).
  - DMA issue order is latency-aware: SDMA engines round-robin across all
    in-flight transfers, so the first two x chunks go as small early
    DMAs, later pairs and the 2MB residual load are issued inside the
    loop; outputs are stored as 8 paired DMAs. (Sync issue cost is
    ~650ns per DMA and a transfer only completes at aggregate-BW share.)
  - A 30-matmul junk warmup bridges the DMA wait so the PE stays busy
    through the HAM activity window and the clock gate opens (1.2 ->
    2.4 GHz) at ~10.6us and stays open, instead of oscillating.
  - Scores run 2-way row-tiled (64-deep contraction = 2 replicated D=32
    bands): tiles (0,0)/(64,0) write different PSUM banks concurrently,
    roughly halving score matmul occupancy; emitted in 3-superstep
    batches (matching the 3 PSUM score slots) to bound tile-mode-switch
    drains.
  - The softmax exp stream is split: 54 supersteps use the ScalarE Exp
    LUT (~1.0us per [128,1024] PSUM tile); 10 are offloaded to VectorE
    via a Schraudolph bit-trick exp writing f8e4m3 bits directly
    (bits8 = 8*log2e*(s-2) + 56, via the HW's saturating
    round-to-nearest f32->uint8 convert), accuracy-neutral next to the
    f8 quantization both exp paths share. ScalarE also absorbs the first
    four W evacuations during its ramp stalls.
  - Epilogue: accumulators are copied out of PSUM immediately (freeing
    the two pa banks for the next group's attend), and the divide +
    residual run one superstep later as a single scalar_tensor_tensor.
  - LAG 16 -> 8: the attend stream trails closer, cutting the drain tail.
"""

import sys

if "/opt/trn_rl_repo" not in sys.path:
    sys.path.insert(0, "/opt/trn_rl_repo")

import numpy as np
import ml_dtypes

BF16 = ml_dtypes.bfloat16
F8 = ml_dtypes.float8_e4m3

# Problem constants
B, HH, WW, C = 4, 64, 64, 256
D = 32
N = HH * WW          # 4096 keys per sample
NQ = N // 2          # 2048 queries per core
NCORES = 8

LAG = 8              # attend trails the scores/exp stream by LAG supersteps
WARMUP_MMS = 30
# Supersteps whose exp runs on VectorE (Schraudolph) instead of ScalarE;
# spaced >=4 apart, away from epilogue supersteps (vv = 8k+7+LAG).
DVE_EXP = frozenset({18, 22, 26, 34, 38, 42, 50, 54, 58, 62})
SCH_A = float(8.0 * np.log2(np.e))             # 11.5416: f8e4m3 bits/ln
SCH_B = float(56.0 - 2.0 * 8.0 * np.log2(np.e))  # f8 bias, shift=2 folded
# phase-B emission: vv -> (qk chunks, w_pass chunks)
PHASE_B = {
    0: ((0, 1), (0,)),
    2: ((2, 3), (1,)),
    4: ((4, 5), (2,)),
    5: ((), (3,)),
    6: ((6, 7), (4,)),
    7: ((), (5,)),
    8: ((), (6,)),
    9: ((), (7,)),
}

_compiled_cache = {}


def _build(use_bias: bool):
    from contextlib import ExitStack
    from concourse import bacc, tile, mybir

    f32 = mybir.dt.float32
    bf = mybir.dt.bfloat16
    f8 = mybir.dt.float8e4
    u8 = mybir.dt.uint8
    DR = mybir.MatmulPerfMode.DoubleRow

    nc = bacc.Bacc("TRN2", target_bir_lowering=False, debug=False, num_devices=NCORES)

    if use_bias:
        xT_d = nc.dram_tensor("xT", [128, 2, N], bf, kind="ExternalInput")
        wb_d = nc.dram_tensor("wblob", [128, 1024], bf, kind="ExternalInput")
        wbias_d = nc.dram_tensor("wbias", [1, 512], bf, kind="ExternalInput")
    else:
        xT8_d = nc.dram_tensor("xT8", [128, 2, N], f8, kind="ExternalInput")
        wb8_d = nc.dram_tensor("wblob8", [128, 2, 512], f8, kind="ExternalInput")
    # residual, pre-transposed on host to [128, 16, 256]
    xq32_d = nc.dram_tensor("xq32", [128, 16, C], f32, kind="ExternalInput")
    # output, host un-permutes [8, 128, 2, 256] -> [2048, 256]
    out_d = nc.dram_tensor("out", [8, 128, 2, C], f32, kind="ExternalOutput")

    Exp = mybir.ActivationFunctionType.Exp
    # undo 2x band replication (64-deep tiled score contraction) + 1/sqrt(D)
    SC2 = float(1.0 / (2.0 * np.sqrt(np.float32(D))))
    Add = mybir.AluOpType.add
    Mult = mybir.AluOpType.mult

    with tile.TileContext(nc) as tc:
        with ExitStack() as ctx:
            const = ctx.enter_context(tc.tile_pool(name="const", bufs=1))
            big = ctx.enter_context(tc.tile_pool(name="big", bufs=1))
            xbp = ctx.enter_context(tc.tile_pool(name="xbp", bufs=4))
            expp = ctx.enter_context(tc.tile_pool(name="expp", bufs=16))
            small = ctx.enter_context(tc.tile_pool(name="small", bufs=2))
            # PSUM: 3 x [128,1024] working tiles (6 banks; shared by phase-B
            # projections and score matmuls) + 2 pa accumulator banks = 8.
            ps_sc = ctx.enter_context(tc.tile_pool(name="ps_sc", bufs=3, space="PSUM"))
            ps_pa = ctx.enter_context(tc.tile_pool(name="ps_pa", bufs=2, space="PSUM"))

            # ---- PE warm-up on junk data while the first DMAs land ----
            wu = const.tile([128, 128], bf, tag="wu")
            nc.gpsimd.memset(wu[:], 1.0)
            pwarm = ps_sc.tile([128, 1024], f32, tag="sc", name="pwarm")
            for _ in range(WARMUP_MMS):
                nc.tensor.matmul(pwarm[:, 0:128], wu[:], wu[:], start=True, stop=True)

            # ---- weights (one blob DMA) ----
            if use_bias:
                wall = const.tile([128, 1024], bf, tag="wall")
                nc.sync.dma_start(out=wall[:], in_=wb_d[:])
                wq0 = wall[:, 0:128]
                wq1 = wall[:, 128:256]
                wk0 = wall[:, 256:384]
                wk1 = wall[:, 384:512]
                wvo0 = wall[:, 512:768]
                wvo1 = wall[:, 768:1024]
                ones_row = const.tile([1, 512], bf, tag="ones_row")
                nc.gpsimd.memset(ones_row[:], 1.0)
                wbias = const.tile([1, 512], bf, tag="wbias")
                nc.sync.dma_start(out=wbias[:], in_=wbias_d[:])
                wqb = wbias[:, 0:128]
                wkb = wbias[:, 128:256]
                wvob = wbias[:, 256:512]
            else:
                w8all = const.tile([128, 2, 512], f8, tag="w8all")
                nc.sync.dma_start(out=w8all[:], in_=wb8_d[:])

            # ---- phase B inputs ----
            # SDMA engines round-robin across ALL in-flight transfers, so
            # the issue order IS the priority order: chunks 0 and 1 go as
            # small single DMAs (low latency), later pairs and the big
            # residual load are issued inside the loop, after the data they
            # would otherwise starve.
            xT_src = xT_d if use_bias else xT8_d
            xdt = bf if use_bias else f8
            xbs = []
            for s in range(2):
                xp = xbp.tile([128, 2, 512], xdt, tag="xbs", bufs=2)
                nc.sync.dma_start(out=xp[:], in_=xT_src[:, :, 512 * s : 512 * s + 512])
                xbs.append(xp[:])
            xpairs = []
            for i in range(1, 4):
                xp = xbp.tile([128, 2, 1024], xdt, tag="xb", bufs=3)
                xpairs.append((xp, i))
                xbs.append(xp[:, :, 0:512])
                xbs.append(xp[:, :, 512:1024])

            # residual x for the epilogue: one DMA, resident in SBUF
            xqall = big.tile([128, 16, C], f32, tag="xqall")

            # Persistent SBUF: qT replicated across the 4 partition bands of
            # 32 (read as 2 bands of 64 by the tiled score matmuls), kT in 2
            # bands of 64 (band 0: chunks 4s/4s+1, band 1: chunks 4s+2/4s+3),
            # and W (= V@Wo) rows with a ones column at 256 for the softmax
            # denominator.
            qT4 = big.tile([128, NQ], bf, tag="qT4")
            kT4 = big.tile([128, N], bf, tag="kT4")
            wsb = big.tile([128, 16, 2, 272], f8, tag="wsb")
            nc.vector.memset(wsb[:, :, :, 256:257], 1.0)
            expbias = const.tile([128, 1], f32, tag="expbias")
            nc.vector.memset(expbias[:], -2.0)

            def qk_chunk(s):
                # q (chunks 0-3) and k share one [128,1024] psum tile
                p = ps_sc.tile([128, 1024], f32, tag="sc", name=f"pqk{s}")
                if use_bias:
                    if s < 4:
                        nc.tensor.matmul(p[:, 0:512], wq0, xbs[s][:, 0, :], start=True, stop=False)
                        nc.tensor.matmul(p[:, 0:512], wq1, xbs[s][:, 1, :], start=False, stop=False)
                        nc.tensor.matmul(p[:, 0:512], wqb, ones_row[:], start=False, stop=True)
                    nc.tensor.matmul(p[:, 512:1024], wk0, xbs[s][:, 0, :], start=True, stop=False)
                    nc.tensor.matmul(p[:, 512:1024], wk1, xbs[s][:, 1, :], start=False, stop=False)
                    nc.tensor.matmul(p[:, 512:1024], wkb, ones_row[:], start=False, stop=True)
                else:
                    if s < 4:
                        nc.tensor.matmul(p[:, 0:512], w8all[:, :, 0:128], xbs[s], start=True, stop=True, perf_mode=DR)
                    nc.tensor.matmul(p[:, 512:1024], w8all[:, :, 128:256], xbs[s], start=True, stop=True, perf_mode=DR)
                # k carries the softmax scale and the 1/2 band-contraction
                # factor; band 0 <- chunks 4s,4s+1 ; band 1 <- 4s+2,4s+3
                # (psum k bands are 4 identical replicas, read diagonally).
                # k first: it gates the score matmuls.
                nc.vector.tensor_scalar(
                    kT4[0:64, 256 * s : 256 * s + 256], p[0:64, 512:768], SC2, None, Mult
                )
                nc.vector.tensor_scalar(
                    kT4[64:128, 256 * s : 256 * s + 256], p[64:128, 768:1024], SC2, None, Mult
                )
                if s < 4:
                    nc.vector.tensor_copy(qT4[:, 512 * s : 512 * s + 512], p[:, 0:512])

            def w_pass(s):
                # W = x @ (wv@wo): 4 key chunks of 128 per xb in one psum tile
                pw = ps_sc.tile([128, 1024], f32, tag="sc", name=f"pw{s}")
                for j2 in range(4):
                    off = 128 * j2
                    if use_bias:
                        nc.tensor.matmul(pw[:, 256 * j2 : 256 * j2 + 256], xbs[s][:, 0, off : off + 128], wvo0, start=True, stop=False)
                        nc.tensor.matmul(pw[:, 256 * j2 : 256 * j2 + 256], xbs[s][:, 1, off : off + 128], wvo1, start=False, stop=False)
                        nc.tensor.matmul(pw[:, 256 * j2 : 256 * j2 + 256], ones_row[:, 0:128], wvob, start=False, stop=True)
                    else:
                        nc.tensor.matmul(pw[:, 256 * j2 : 256 * j2 + 256], xbs[s][:, :, off : off + 128], w8all[:, :, 256:512], start=True, stop=True, perf_mode=DR)
                # early W evacuations ride the ScalarE (its ramp exps are
                # chain-stalled anyway, and they were clogging the DVE queue
                # ahead of the k evacuations); later ones stay on VectorE.
                if s < 4 and not use_bias:
                    nc.scalar.copy(wsb[:, 2 * s : 2 * s + 2, :, 0:256], pw[:])
                else:
                    nc.vector.tensor_copy(wsb[:, 2 * s : 2 * s + 2, :, 0:256], pw[:])

            def scores_mms(g, u):
                # 2-way row tiling: T0 (sbuf partitions 0-63) does chunks
                # 4u,4u+1 into bank A; T8 (64-127) does 4u+2,4u+3 into bank
                # B. Interleaved emission for cross-tile concurrency.
                pst = ps_sc.tile([128, 1024], f32, tag="sc", name=f"ps{g}_{u}")
                qs = slice(256 * g, 256 * g + 256)
                k0 = 256 * u
                nc.tensor.matmul(pst[:, 0:256], kT4[0:64, k0 : k0 + 128], qT4[0:64, qs], start=True, stop=True, tile_position=(0, 0))
                nc.tensor.matmul(pst[:, 512:768], kT4[64:128, k0 : k0 + 128], qT4[64:128, qs], start=True, stop=True, tile_position=(64, 0))
                nc.tensor.matmul(pst[:, 256:512], kT4[0:64, k0 + 128 : k0 + 256], qT4[0:64, qs], start=True, stop=True, tile_position=(0, 0))
                nc.tensor.matmul(pst[:, 768:1024], kT4[64:128, k0 + 128 : k0 + 256], qT4[64:128, qs], start=True, stop=True, tile_position=(64, 0))
                return pst

            def epilogue_evac(g, pa_a, pa_b):
                # Evacuate the accumulators fast so the next group's attend
                # matmuls can reuse the two pa PSUM banks; the divide +
                # residual happen off the critical path a superstep later.
                pasb = small.tile([128, 2, 257], f32, tag="pasb", bufs=2)
                nc.vector.tensor_copy(pasb[:, 0, :], pa_a[:, 0:257])
                nc.vector.tensor_copy(pasb[:, 1, :], pa_b[:, 0:257])
                return pasb

            def epilogue_tail(g, pasb):
                ot = small.tile([128, 2, C], f32, tag="ot", bufs=3)
                for h in range(2):
                    qb = 2 * g + h
                    rec = small.tile([128, 1], f32, tag="rec")
                    nc.vector.reciprocal(rec[:], pasb[:, h, 256:257])
                    # out = attended/denom + x in one DVE op
                    nc.vector.scalar_tensor_tensor(
                        ot[:, h, :], pasb[:, h, 0:256], rec[:], xqall[:, qb, :], Mult, Add
                    )
                nc.sync.dma_start(out=out_d[g], in_=ot[:])

            # ---- software pipeline ----
            # scores/exp stream order: chunk-diagonal over groups 0-1 during
            # the ramp (both only need q chunk 0 + k chunk u), then
            # group-major. Scores are emitted in PAIRS of supersteps (even
            # vv) so the PE switches tile mode at most twice per pair.
            pa_tiles = {}
            ets = {}
            pending_scores = {}
            pending_epi = []
            NSS = 8 * 8
            sched_scores = [(g, u) for u in range(8) for g in (0, 1)] + [
                (g, u) for g in range(2, 8) for u in range(8)
            ]
            sched_attend = [(g, u) for g in range(8) for u in range(8)]

            def do_exp(vv):
                g, u = sched_scores[vv]
                pst = pending_scores.pop(vv)
                et = expp.tile([128, 2, 2, 256], f8, tag="e")
                if vv in DVE_EXP and not use_bias:
                    # Split exp across both engines: ScalarE takes PSUM bank
                    # A's half via the Exp LUT (it is stall-idle at these
                    # supersteps anyway), VectorE takes bank B via the
                    # Schraudolph bit-trick exp straight to f8 bits
                    # (bits8 = A*s + B; the HW f32->uint8 convert rounds to
                    # nearest-even and SATURATES [0,255], so underflow lands
                    # exactly on f8 zero). Halving the DVE insertion frees
                    # the score PSUM slot sooner.
                    nc.scalar.activation(et[:, 0, :, :], pst[:, 0:512], Exp, bias=expbias[:])
                    nc.vector.tensor_scalar(et[:, 1, :, :].bitcast(u8), pst[:, 512:1024], SCH_A, SCH_B, Mult, Add)
                else:
                    # exp(s - 2): constant shift keeps exp within fp8-e4m3
                    # range (max score ~7 -> e^5 = 148 < 240); the
                    # ones-column denominator sees the same shift, so the
                    # normalized ratio is exact. The first two supersteps
                    # run as two half-tile ACTIVATEs so the exp stream
                    # starts as soon as the T0 score pair (PSUM bank A) is
                    # done, without waiting for the T8 pair.
                    if vv < 2:
                        nc.scalar.activation(et[:, 0, :, :], pst[:, 0:512], Exp, bias=expbias[:])
                        nc.scalar.activation(et[:, 1, :, :], pst[:, 512:1024], Exp, bias=expbias[:])
                    else:
                        nc.scalar.activation(et[:], pst[:], Exp, bias=expbias[:])
                ets[(g, u)] = et

            for vv in range(NSS + LAG):
                if vv == 0:
                    xp, i = xpairs[0]
                    nc.sync.dma_start(out=xp[:], in_=xT_src[:, :, 1024 * i : 1024 * i + 1024])
                elif vv == 2:
                    xp, i = xpairs[1]
                    nc.sync.dma_start(out=xp[:], in_=xT_src[:, :, 1024 * i : 1024 * i + 1024])
                elif vv == 4:
                    xp, i = xpairs[2]
                    nc.sync.dma_start(out=xp[:], in_=xT_src[:, :, 1024 * i : 1024 * i + 1024])
                elif vv == 6:
                    nc.sync.dma_start(out=xqall[:], in_=xq32_d[:])
                qks, ws = PHASE_B.get(vv, ((), ()))
                while pending_epi and pending_epi[0][0] != (vv - 1 - LAG) // 8:
                    g_e, pasb_e = pending_epi.pop(0)
                    epilogue_tail(g_e, pasb_e)
                for s in qks:
                    qk_chunk(s)
                for s in ws:
                    w_pass(s)
                if vv < NSS:
                    if vv % 3 == 0:
                        for v2 in range(vv, min(vv + 3, NSS)):
                            pending_scores[v2] = scores_mms(*sched_scores[v2])
                    do_exp(vv)
                va = vv - LAG
                if 0 <= va < NSS:
                    g_p, u_p = sched_attend[va]
                    if u_p == 0:
                        pa_tiles[2 * g_p] = ps_pa.tile([128, 512], f32, tag="pa", name=f"pa{2 * g_p}")
                        pa_tiles[2 * g_p + 1] = ps_pa.tile([128, 512], f32, tag="pa", name=f"pa{2 * g_p + 1}")
                    et_p = ets.pop((g_p, u_p))
                    for jp in range(2):
                        P = 2 * u_p + jp  # wsb pair: key chunks 2P, 2P+1
                        for h in range(2):
                            nc.tensor.matmul(
                                pa_tiles[2 * g_p + h][:, 0:257],
                                et_p[:, jp, :, 128 * h : 128 * h + 128],
                                wsb[:, P, :, 0:257],
                                start=(P == 0),
                                stop=(P == 15),
                                perf_mode=DR,
                            )
                    if u_p == 7:
                        if g_p == 7:
                            # final group: nothing reuses the pa banks, so
                            # skip the evacuation hop and write out directly
                            ot7 = small.tile([128, 2, C], f32, tag="ot", bufs=3)
                            for h in range(2):
                                rec7 = small.tile([128, 1], f32, tag="rec")
                                nc.vector.reciprocal(rec7[:], pa_tiles[2 * g_p + h][:, 256:257])
                                nc.vector.scalar_tensor_tensor(
                                    ot7[:, h, :], pa_tiles[2 * g_p + h][:, 0:256], rec7[:],
                                    xqall[:, 2 * g_p + h, :], Mult, Add,
                                )
                            nc.sync.dma_start(out=out_d[g_p], in_=ot7[:])
                        else:
                            pending_epi.append((g_p, epilogue_evac(g_p, pa_tiles[2 * g_p], pa_tiles[2 * g_p + 1])))
                        del pa_tiles[2 * g_p]
                        del pa_tiles[2 * g_p + 1]
            for g_e, pasb_e in pending_epi:
                epilogue_tail(g_e, pasb_e)

    nc.compile()
    return nc


def _get_compiled(use_bias: bool):
    key = bool(use_bias)
    if key not in _compiled_cache:
        _compiled_cache[key] = _build(use_bias)
    return _compiled_cache[key]


def _prep(x, wq, bq, wk, bk, wv, bv, wo, bo):
    xf = np.ascontiguousarray(np.asarray(x, dtype=np.float32)).reshape(B, N, C)
    wq = np.asarray(wq, np.float32)
    bq = np.asarray(bq, np.float32)
    wk = np.asarray(wk, np.float32)
    bk = np.asarray(bk, np.float32)
    wv = np.asarray(wv, np.float32)
    bv = np.asarray(bv, np.float32)
    wo = np.asarray(wo, np.float32)
    bo = np.asarray(bo, np.float32)

    use_bias = not (
        np.all(bq == 0) and np.all(bk == 0) and np.all(bv == 0) and np.all(bo == 0)
    )

    # Weights go in UNSCALED; the softmax scale and the band-contraction
    # factor are applied at the k evacuation.
    wvo = (wv @ wo).astype(BF16)  # fold wo into the value projection
    if use_bias:
        wq_rep = np.tile(wq, (1, 4)).astype(BF16)  # [256, 128]
        wk_rep = np.tile(wk, (1, 4)).astype(BF16)
        wblob = np.ascontiguousarray(
            np.concatenate(
                [wq_rep[0:128], wq_rep[128:256], wk_rep[0:128], wk_rep[128:256],
                 wvo[0:128], wvo[128:256]],
                axis=1,
            )
        )  # [128, 1024]
        wbias = np.ascontiguousarray(
            np.concatenate(
                [np.tile(bq, 4), np.tile(bk, 4), bv @ wo], 0
            )[None, :]
        ).astype(BF16)  # [1, 512]
    else:
        # fp8 DoubleRow packing: [ki, ko, col] = w[ki + 128*ko, col]
        wq_rep = np.tile(wq, (1, 4)).astype(np.float32)
        wk_rep = np.tile(wk, (1, 4)).astype(np.float32)
        blob = np.concatenate([wq_rep, wk_rep, wvo.astype(np.float32)], axis=1)  # [256, 512]
        wblob8 = np.ascontiguousarray(
            blob.reshape(2, 128, 512).transpose(1, 0, 2).astype(F8)
        )  # [128, 2, 512]

    in_maps = []
    for core in range(NCORES):
        b, h = divmod(core, 2)
        if h == 0:
            xo = xf[b]
        else:
            xo = np.concatenate([xf[b, NQ:], xf[b, :NQ]], 0)
        # channel-major transpose on host: [256, 4096] -> [128, 2, 4096]
        xT = np.ascontiguousarray(
            xo.T.reshape(2, 128, N).transpose(1, 0, 2)
        )
        xq = np.ascontiguousarray(xo[:NQ])
        if use_bias:
            xq = xq + bo[None, :]
        # residual pre-transposed to [128 partitions, 16 chunks, 256]
        xqt = np.ascontiguousarray(xq.reshape(16, 128, C).transpose(1, 0, 2))
        if use_bias:
            im = {"xT": xT.astype(BF16), "xq32": xqt, "wblob": wblob, "wbias": wbias}
        else:
            im = {"xT8": xT.astype(F8), "xq32": xqt, "wblob8": wblob8}
        in_maps.append(im)
    return in_maps, use_bias


def _gather(results):
    out = np.empty((B, N, C), np.float32)
    for core in range(NCORES):
        b, h = divmod(core, 2)
        # device layout [8 groups, 128, 2, 256] -> [2048, 256]
        o = results[core]["out"].reshape(8, 128, 2, C).transpose(0, 2, 1, 3).reshape(NQ, C)
        out[b, NQ * h : NQ * (h + 1)] = o
    return out.reshape(B, HH, WW, C)


def kernel(x, wq, bq, wk, bk, wv, bv, wo, bo):
    from concourse.bass_utils import run_bass_kernel_spmd

    in_maps, use_bias = _prep(x, wq, bq, wk, bk, wv, bv, wo, bo)
    nc = _get_compiled(use_bias)
    res = run_bass_kernel_spmd(nc, in_maps, core_ids=list(range(NCORES)))
    return _gather(res.results)


def _ensure_ntff_hook():
    """The agent image's antenv stub lacks axon_hooks; synthesize it so
    run_bass_kernel_spmd(trace=True) can NTFF-profile via libaxon_pjrt."""
    import types

    try:
        from antenv.axon_hooks import get_axon_ntff_profile_hook  # noqa: F401
        return
    except ImportError:
        pass
    import antenv
    from trn_agent_boot.trn_boot import _ntff_profile_via_ctypes

    mod = types.ModuleType("antenv.axon_hooks")
    state = {"h": _ntff_profile_via_ctypes("/opt/axon/libaxon_pjrt.so")}
    mod.get_axon_ntff_profile_hook = lambda: state["h"]
    mod.set_axon_ntff_profile_hook = lambda h: state.__setitem__("h", h)
    sys.modules["antenv.axon_hooks"] = mod
    antenv.axon_hooks = mod


def run_traced(inputs, **kw):
    """For test.py: run with NTFF profiling; returns (output, BassKernelResults)."""
    from concourse.bass_utils import run_bass_kernel_spmd

    _ensure_ntff_hook()

    in_maps, use_bias = _prep(**inputs)
    nc = _get_compiled(use_bias)
    res = run_bass_kernel_spmd(nc, in_maps, core_ids=list(range(NCORES)), trace=True, **kw)
    return _gather(res.results), res


# revision 33
# speedup vs baseline: 1.0543x; 1.0508x over previous
"""Trainium2 Bass kernel for nn_AttentionBlock (B=4, H=W=64, C=256, D=32).

Sharding: 8 shards = 4 samples x 2 query-halves. Each core gets the full
sample (rows reordered so its 2048 query rows come first), computes K and
the fused V@Wo projection for all 4096 keys, and attention + residual for
its 2048 queries. No collectives needed.

Over the previous 98.1us version (measured ~90.6us):
  - Phase B (q/k and W=x@(wv@wo) projections) runs fp8-e4m3 DoubleRow: x
    arrives as one f8 [128,2,4096] tensor (half the DMA), each projection
    is a single DR matmul per chunk (half the PE time).
  - DMA issue order is latency-aware: SDMA engines round-robin across all
    in-flight transfers, so the first two x chunks go as small early
    DMAs, later pairs and the 2MB residual load are issued inside the
    loop; outputs are stored as 8 paired DMAs. (Sync issue cost is
    ~650ns per DMA and a transfer only completes at aggregate-BW share.)
  - A 30-matmul junk warmup bridges the DMA wait so the PE stays busy
    through the HAM activity window and the clock gate opens (1.2 ->
    2.4 GHz) at ~10.6us and stays open, instead of oscillating.
  - Scores run 2-way row-tiled (64-deep contraction = 2 replicated D=32
    bands): tiles (0,0)/(64,0) write different PSUM banks concurrently,
    roughly halving score matmul occupancy; emitted in 3-superstep
    batches (matching the 3 PSUM score slots) to bound tile-mode-switch
    drains.
  - The softmax exp stream is split: 54 supersteps use the ScalarE Exp
    LUT (~1.0us per [128,1024] PSUM tile); 10 are offloaded to VectorE
    via a Schraudolph bit-trick exp writing f8e4m3 bits directly
    (bits8 = 8*log2e*(s-2) + 56, via the HW's saturating
    round-to-nearest f32->uint8 convert), accuracy-neutral next to the
    f8 quantization both exp paths share. ScalarE also absorbs the first
    four W evacuations during its ramp stalls.
  - Epilogue: accumulators are copied out of PSUM immediately (freeing
    the two pa banks for the next group's attend), and the divide +
    residual run one superstep later as a single scalar_tensor_tensor.
  - LAG 16 -> 8: the attend stream trails closer, cutting the drain tail.
"""

import sys

if "/opt/trn_rl_repo" not in sys.path:
    sys.path.insert(0, "/opt/trn_rl_repo")

import numpy as np
import ml_dtypes

BF16 = ml_dtypes.bfloat16
F8 = ml_dtypes.float8_e4m3

# Problem constants
B, HH, WW, C = 4, 64, 64, 256
D = 32
N = HH * WW          # 4096 keys per sample
NQ = N // 2          # 2048 queries per core
NCORES = 8

LAG = 8              # attend trails the scores/exp stream by LAG supersteps
WARMUP_MMS = 30
# Supersteps whose exp runs on VectorE (Schraudolph) instead of ScalarE;
# spaced >=4 apart, away from epilogue supersteps (vv = 8k+7+LAG).
DVE_EXP = frozenset({18, 22, 26, 34, 38, 42, 50, 54, 58, 62})
SCH_A = float(8.0 * np.log2(np.e))             # 11.5416: f8e4m3 bits/ln
SCH_B = float(56.0 - 2.0 * 8.0 * np.log2(np.e))  # f8 bias, shift=2 folded
# phase-B emission: vv -> (qk chunks, w_pass chunks)
PHASE_B = {
    0: ((0, 1), (0,)),
    2: ((2, 3), (1,)),
    4: ((4, 5), (2,)),
    5: ((), (3,)),
    6: ((6, 7), (4,)),
    7: ((), (5,)),
    8: ((), (6,)),
    9: ((), (7,)),
}

_compiled_cache = {}


def _build(use_bias: bool):
    from contextlib import ExitStack
    from concourse import bacc, tile, mybir

    f32 = mybir.dt.float32
    bf = mybir.dt.bfloat16
    f8 = mybir.dt.float8e4
    u8 = mybir.dt.uint8
    DR = mybir.MatmulPerfMode.DoubleRow

    nc = bacc.Bacc("TRN2", target_bir_lowering=False, debug=False, num_devices=NCORES)

    if use_bias:
        xT_d = nc.dram_tensor("xT", [128, 2, N], bf, kind="ExternalInput")
        wb_d = nc.dram_tensor("wblob", [128, 1024], bf, kind="ExternalInput")
        wbias_d = nc.dram_tensor("wbias", [1, 512], bf, kind="ExternalInput")
    else:
        xT8_d = nc.dram_tensor("xT8", [128, 2, N], f8, kind="ExternalInput")
        wb8_d = nc.dram_tensor("wblob8", [128, 2, 512], f8, kind="ExternalInput")
    # residual, pre-transposed on host to [128, 16, 256]
    xq32_d = nc.dram_tensor("xq32", [128, 16, C], f32, kind="ExternalInput")
    # output, host un-permutes [8, 128, 2, 256] -> [2048, 256]
    out_d = nc.dram_tensor("out", [8, 128, 2, C], f32, kind="ExternalOutput")

    Exp = mybir.ActivationFunctionType.Exp
    # undo 2x band replication (64-deep tiled score contraction) + 1/sqrt(D)
    SC2 = float(1.0 / (2.0 * np.sqrt(np.float32(D))))
    Add = mybir.AluOpType.add
    Mult = mybir.AluOpType.mult

    with tile.TileContext(nc) as tc:
        with ExitStack() as ctx:
            const = ctx.enter_context(tc.tile_pool(name="const", bufs=1))
            big = ctx.enter_context(tc.tile_pool(name="big", bufs=1))
            xbp = ctx.enter_context(tc.tile_pool(name="xbp", bufs=4))
            expp = ctx.enter_context(tc.tile_pool(name="expp", bufs=16))
            small = ctx.enter_context(tc.tile_pool(name="small", bufs=2))
            # PSUM: 3 x [128,1024] working tiles (6 banks; shared by phase-B
            # projections and score matmuls) + 2 pa accumulator banks = 8.
            ps_sc = ctx.enter_context(tc.tile_pool(name="ps_sc", bufs=3, space="PSUM"))
            ps_pa = ctx.enter_context(tc.tile_pool(name="ps_pa", bufs=2, space="PSUM"))

            # ---- PE warm-up on junk data while the first DMAs land ----
            wu = const.tile([128, 128], bf, tag="wu")
            nc.gpsimd.memset(wu[:], 1.0)
            pwarm = ps_sc.tile([128, 1024], f32, tag="sc", name="pwarm")
            for _ in range(WARMUP_MMS):
                nc.tensor.matmul(pwarm[:, 0:128], wu[:], wu[:], start=True, stop=True)

            # ---- weights (one blob DMA) ----
            if use_bias:
                wall = const.tile([128, 1024], bf, tag="wall")
                nc.sync.dma_start(out=wall[:], in_=wb_d[:])
                wq0 = wall[:, 0:128]
                wq1 = wall[:, 128:256]
                wk0 = wall[:, 256:384]
                wk1 = wall[:, 384:512]
                wvo0 = wall[:, 512:768]
                wvo1 = wall[:, 768:1024]
                ones_row = const.tile([1, 512], bf, tag="ones_row")
                nc.gpsimd.memset(ones_row[:], 1.0)
                wbias = const.tile([1, 512], bf, tag="wbias")
                nc.sync.dma_start(out=wbias[:], in_=wbias_d[:])
                wqb = wbias[:, 0:128]
                wkb = wbias[:, 128:256]
                wvob = wbias[:, 256:512]
            else:
                w8all = const.tile([128, 2, 512], f8, tag="w8all")
                nc.sync.dma_start(out=w8all[:], in_=wb8_d[:])

            # ---- phase B inputs ----
            # SDMA engines round-robin across ALL in-flight transfers, so
            # the issue order IS the priority order: chunks 0 and 1 go as
            # small single DMAs (low latency), later pairs and the big
            # residual load are issued inside the loop, after the data they
            # would otherwise starve.
            xT_src = xT_d if use_bias else xT8_d
            xdt = bf if use_bias else f8
            xbs = []
            for s in range(2):
                xp = xbp.tile([128, 2, 512], xdt, tag="xbs", bufs=2)
                nc.sync.dma_start(out=xp[:], in_=xT_src[:, :, 512 * s : 512 * s + 512])
                xbs.append(xp[:])
            xpairs = []
            for i in range(1, 4):
                xp = xbp.tile([128, 2, 1024], xdt, tag="xb", bufs=3)
                xpairs.append((xp, i))
                xbs.append(xp[:, :, 0:512])
                xbs.append(xp[:, :, 512:1024])

            # residual x for the epilogue: one DMA, resident in SBUF
            xqall = big.tile([128, 16, C], f32, tag="xqall")

            # Persistent SBUF: qT replicated across the 4 partition bands of
            # 32 (read as 2 bands of 64 by the tiled score matmuls), kT in 2
            # bands of 64 (band 0: chunks 4s/4s+1, band 1: chunks 4s+2/4s+3),
            # and W (= V@Wo) rows with a ones column at 256 for the softmax
            # denominator.
            qT4 = big.tile([128, NQ], bf, tag="qT4")
            kT4 = big.tile([128, N], bf, tag="kT4")
            wsb = big.tile([128, 16, 2, 272], f8, tag="wsb")
            nc.vector.memset(wsb[:, :, :, 256:257], 1.0)
            expbias = const.tile([128, 1], f32, tag="expbias")
            nc.vector.memset(expbias[:], -2.0)

            def qk_chunk(s):
                # q (chunks 0-3) and k share one [128,1024] psum tile
                p = ps_sc.tile([128, 1024], f32, tag="sc", name=f"pqk{s}")
                if use_bias:
                    if s < 4:
                        nc.tensor.matmul(p[:, 0:512], wq0, xbs[s][:, 0, :], start=True, stop=False)
                        nc.tensor.matmul(p[:, 0:512], wq1, xbs[s][:, 1, :], start=False, stop=False)
                        nc.tensor.matmul(p[:, 0:512], wqb, ones_row[:], start=False, stop=True)
                    nc.tensor.matmul(p[:, 512:1024], wk0, xbs[s][:, 0, :], start=True, stop=False)
                    nc.tensor.matmul(p[:, 512:1024], wk1, xbs[s][:, 1, :], start=False, stop=False)
                    nc.tensor.matmul(p[:, 512:1024], wkb, ones_row[:], start=False, stop=True)
                else:
                    if s < 4:
                        nc.tensor.matmul(p[:, 0:512], w8all[:, :, 0:128], xbs[s], start=True, stop=True, perf_mode=DR)
                    nc.tensor.matmul(p[:, 512:1024], w8all[:, :, 128:256], xbs[s], start=True, stop=True, perf_mode=DR)
                # k carries the softmax scale and the 1/2 band-contraction
                # factor; band 0 <- chunks 4s,4s+1 ; band 1 <- 4s+2,4s+3
                # (psum k bands are 4 identical replicas, read diagonally).
                # k first: it gates the score matmuls.
                nc.vector.tensor_scalar(
                    kT4[0:64, 256 * s : 256 * s + 256], p[0:64, 512:768], SC2, None, Mult
                )
                nc.vector.tensor_scalar(
                    kT4[64:128, 256 * s : 256 * s + 256], p[64:128, 768:1024], SC2, None, Mult
                )
                if s < 4:
                    nc.vector.tensor_copy(qT4[:, 512 * s : 512 * s + 512], p[:, 0:512])

            def w_pass(s):
                # W = x @ (wv@wo): 4 key chunks of 128 per xb in one psum tile
                pw = ps_sc.tile([128, 1024], f32, tag="sc", name=f"pw{s}")
                for j2 in range(4):
                    off = 128 * j2
                    if use_bias:
                        nc.tensor.matmul(pw[:, 256 * j2 : 256 * j2 + 256], xbs[s][:, 0, off : off + 128], wvo0, start=True, stop=False)
                        nc.tensor.matmul(pw[:, 256 * j2 : 256 * j2 + 256], xbs[s][:, 1, off : off + 128], wvo1, start=False, stop=False)
                        nc.tensor.matmul(pw[:, 256 * j2 : 256 * j2 + 256], ones_row[:, 0:128], wvob, start=False, stop=True)
                    else:
                        nc.tensor.matmul(pw[:, 256 * j2 : 256 * j2 + 256], xbs[s][:, :, off : off + 128], w8all[:, :, 256:512], start=True, stop=True, perf_mode=DR)
                # early W evacuations ride the ScalarE (its ramp exps are
                # chain-stalled anyway, and they were clogging the DVE queue
                # ahead of the k evacuations); later ones stay on VectorE.
                if s < 4 and not use_bias:
                    nc.scalar.copy(wsb[:, 2 * s : 2 * s + 2, :, 0:256], pw[:])
                else:
                    nc.vector.tensor_copy(wsb[:, 2 * s : 2 * s + 2, :, 0:256], pw[:])

            def scores_mms(g, u):
                # 2-way row tiling: T0 (sbuf partitions 0-63) does chunks
                # 4u,4u+1 into bank A; T8 (64-127) does 4u+2,4u+3 into bank
                # B. Interleaved emission for cross-tile concurrency.
                pst = ps_sc.tile([128, 1024], f32, tag="sc", name=f"ps{g}_{u}")
                qs = slice(256 * g, 256 * g + 256)
                k0 = 256 * u
                nc.tensor.matmul(pst[:, 0:256], kT4[0:64, k0 : k0 + 128], qT4[0:64, qs], start=True, stop=True, tile_position=(0, 0))
                nc.tensor.matmul(pst[:, 512:768], kT4[64:128, k0 : k0 + 128], qT4[64:128, qs], start=True, stop=True, tile_position=(64, 0))
                nc.tensor.matmul(pst[:, 256:512], kT4[0:64, k0 + 128 : k0 + 256], qT4[0:64, qs], start=True, stop=True, tile_position=(0, 0))
                nc.tensor.matmul(pst[:, 768:1024], kT4[64:128, k0 + 128 : k0 + 256], qT4[64:128, qs], start=True, stop=True, tile_position=(64, 0))
                return pst

            def epilogue_evac(g, pa_a, pa_b):
                # Evacuate the accumulators fast so the next group's attend
                # matmuls can reuse the two pa PSUM banks; the divide +
                # residual happen off the critical path a superstep later.
                pasb = small.tile([128, 2, 257], f32, tag="pasb", bufs=2)
                nc.vector.tensor_copy(pasb[:, 0, :], pa_a[:, 0:257])
                nc.vector.tensor_copy(pasb[:, 1, :], pa_b[:, 0:257])
                return pasb

            def epilogue_tail(g, pasb):
                ot = small.tile([128, 2, C], f32, tag="ot", bufs=3)
                for h in range(2):
                    qb = 2 * g + h
                    rec = small.tile([128, 1], f32, tag="rec")
                    nc.vector.reciprocal(rec[:], pasb[:, h, 256:257])
                    # out = attended/denom + x in one DVE op
                    nc.vector.scalar_tensor_tensor(
                        ot[:, h, :], pasb[:, h, 0:256], rec[:], xqall[:, qb, :], Mult, Add
                    )
                nc.sync.dma_start(out=out_d[g], in_=ot[:])

            # ---- software pipeline ----
            # scores/exp stream order: chunk-diagonal over groups 0-1 during
            # the ramp (both only need q chunk 0 + k chunk u), then
            # group-major. Scores are emitted in PAIRS of supersteps (even
            # vv) so the PE switches tile mode at most twice per pair.
            pa_tiles = {}
            ets = {}
            pending_scores = {}
            pending_epi = []
            NSS = 8 * 8
            sched_scores = [(g, u) for u in range(8) for g in (0, 1)] + [
                (g, u) for g in range(2, 8) for u in range(8)
            ]
            sched_attend = [(g, u) for g in range(8) for u in range(8)]

            def do_exp(vv):
                g, u = sched_scores[vv]
                pst = pending_scores.pop(vv)
                et = expp.tile([128, 2, 2, 256], f8, tag="e")
                if vv in DVE_EXP and not use_bias:
                    # Schraudolph exp on VectorE, straight to f8 bits:
                    # bits8 = A*s + B (the -2 shift folded into B); the HW
                    # f32->uint8 convert rounds-to-nearest-even and
                    # SATURATES [0,255], so score underflow lands exactly on
                    # f8 zero. The approximation error is below the f8
                    # quantization both exp paths share.
                    nc.vector.tensor_scalar(et[:].bitcast(u8), pst[:], SCH_A, SCH_B, Mult, Add)
                else:
                    # exp(s - 2): constant shift keeps exp within fp8-e4m3
                    # range (max score ~7 -> e^5 = 148 < 240); the
                    # ones-column denominator sees the same shift, so the
                    # normalized ratio is exact. The first two supersteps
                    # run as two half-tile ACTIVATEs so the exp stream
                    # starts as soon as the T0 score pair (PSUM bank A) is
                    # done, without waiting for the T8 pair.
                    if vv < 2:
                        nc.scalar.activation(et[:, 0, :, :], pst[:, 0:512], Exp, bias=expbias[:])
                        nc.scalar.activation(et[:, 1, :, :], pst[:, 512:1024], Exp, bias=expbias[:])
                    else:
                        nc.scalar.activation(et[:], pst[:], Exp, bias=expbias[:])
                ets[(g, u)] = et

            for vv in range(NSS + LAG):
                if vv == 0:
                    xp, i = xpairs[0]
                    nc.sync.dma_start(out=xp[:], in_=xT_src[:, :, 1024 * i : 1024 * i + 1024])
                elif vv == 2:
                    xp, i = xpairs[1]
                    nc.sync.dma_start(out=xp[:], in_=xT_src[:, :, 1024 * i : 1024 * i + 1024])
                elif vv == 4:
                    xp, i = xpairs[2]
                    nc.sync.dma_start(out=xp[:], in_=xT_src[:, :, 1024 * i : 1024 * i + 1024])
                elif vv == 6:
                    nc.sync.dma_start(out=xqall[:], in_=xq32_d[:])
                qks, ws = PHASE_B.get(vv, ((), ()))
                while pending_epi and pending_epi[0][0] != (vv - 1 - LAG) // 8:
                    g_e, pasb_e = pending_epi.pop(0)
                    epilogue_tail(g_e, pasb_e)
                for s in qks:
                    qk_chunk(s)
                for s in ws:
                    w_pass(s)
                if vv < NSS:
                    if vv % 3 == 0:
                        for v2 in range(vv, min(vv + 3, NSS)):
                            pending_scores[v2] = scores_mms(*sched_scores[v2])
                    do_exp(vv)
                va = vv - LAG
                if 0 <= va < NSS:
                    g_p, u_p = sched_attend[va]
                    if u_p == 0:
                        pa_tiles[2 * g_p] = ps_pa.tile([128, 512], f32, tag="pa", name=f"pa{2 * g_p}")
                        pa_tiles[2 * g_p + 1] = ps_pa.tile([128, 512], f32, tag="pa", name=f"pa{2 * g_p + 1}")
                    et_p = ets.pop((g_p, u_p))
                    for jp in range(2):
                        P = 2 * u_p + jp  # wsb pair: key chunks 2P, 2P+1
                        for h in range(2):
                            nc.tensor.matmul(
                                pa_tiles[2 * g_p + h][:, 0:257],
                                et_p[:, jp, :, 128 * h : 128 * h + 128],
                                wsb[:, P, :, 0:257],
                                start=(P == 0),
                                stop=(P == 15),
                                perf_mode=DR,
                            )
                    if u_p == 7:
                        if g_p == 7:
                            # final group: nothing reuses the pa banks, so
                            # skip the evacuation hop and write out directly
                            ot7 = small.tile([128, 2, C], f32, tag="ot", bufs=3)
                            for h in range(2):
                                rec7 = small.tile([128, 1], f32, tag="rec")
                                nc.vector.reciprocal(rec7[:], pa_tiles[2 * g_p + h][:, 256:257])
                                nc.vector.scalar_tensor_tensor(
                                    ot7[:, h, :], pa_tiles[2 * g_p + h][:, 0:256], rec7[:],
                                    xqall[:, 2 * g_p + h, :], Mult, Add,
                                )
                            nc.sync.dma_start(out=out_d[g_p], in_=ot7[:])
                        else:
                            pending_epi.append((g_p, epilogue_evac(g_p, pa_tiles[2 * g_p], pa_tiles[2 * g_p + 1])))
                        del pa_tiles[2 * g_p]
                        del pa_tiles[2 * g_p + 1]
            for g_e, pasb_e in pending_epi:
                epilogue_tail(g_e, pasb_e)

    nc.compile()
    return nc


def _get_compiled(use_bias: bool):
    key = bool(use_bias)
    if key not in _compiled_cache:
        _compiled_cache[key] = _build(use_bias)
    return _compiled_cache[key]


def _prep(x, wq, bq, wk, bk, wv, bv, wo, bo):
    xf = np.ascontiguousarray(np.asarray(x, dtype=np.float32)).reshape(B, N, C)
    wq = np.asarray(wq, np.float32)
    bq = np.asarray(bq, np.float32)
    wk = np.asarray(wk, np.float32)
    bk = np.asarray(bk, np.float32)
    wv = np.asarray(wv, np.float32)
    bv = np.asarray(bv, np.float32)
    wo = np.asarray(wo, np.float32)
    bo = np.asarray(bo, np.float32)

    use_bias = not (
        np.all(bq == 0) and np.all(bk == 0) and np.all(bv == 0) and np.all(bo == 0)
    )

    # Weights go in UNSCALED; the softmax scale and the band-contraction
    # factor are applied at the k evacuation.
    wvo = (wv @ wo).astype(BF16)  # fold wo into the value projection
    if use_bias:
        wq_rep = np.tile(wq, (1, 4)).astype(BF16)  # [256, 128]
        wk_rep = np.tile(wk, (1, 4)).astype(BF16)
        wblob = np.ascontiguousarray(
            np.concatenate(
                [wq_rep[0:128], wq_rep[128:256], wk_rep[0:128], wk_rep[128:256],
                 wvo[0:128], wvo[128:256]],
                axis=1,
            )
        )  # [128, 1024]
        wbias = np.ascontiguousarray(
            np.concatenate(
                [np.tile(bq, 4), np.tile(bk, 4), bv @ wo], 0
            )[None, :]
        ).astype(BF16)  # [1, 512]
    else:
        # fp8 DoubleRow packing: [ki, ko, col] = w[ki + 128*ko, col]
        wq_rep = np.tile(wq, (1, 4)).astype(np.float32)
        wk_rep = np.tile(wk, (1, 4)).astype(np.float32)
        blob = np.concatenate([wq_rep, wk_rep, wvo.astype(np.float32)], axis=1)  # [256, 512]
        wblob8 = np.ascontiguousarray(
            blob.reshape(2, 128, 512).transpose(1, 0, 2).astype(F8)
        )  # [128, 2, 512]

    in_maps = []
    for core in range(NCORES):
        b, h = divmod(core, 2)
        if h == 0:
            xo = xf[b]
        else:
            xo = np.concatenate([xf[b, NQ:], xf[b, :NQ]], 0)
        # channel-major transpose on host: [256, 4096] -> [128, 2, 4096]
        xT = np.ascontiguousarray(
            xo.T.reshape(2, 128, N).transpose(1, 0, 2)
        )
        xq = np.ascontiguousarray(xo[:NQ])
        if use_bias:
            xq = xq + bo[None, :]
        # residual pre-transposed to [128 partitions, 16 chunks, 256]
        xqt = np.ascontiguousarray(xq.reshape(16, 128, C).transpose(1, 0, 2))
        if use_bias:
            im = {"xT": xT.astype(BF16), "xq32": xqt, "wblob": wblob, "wbias": wbias}
        else:
            im = {"xT8": xT.astype(F8), "xq32": xqt, "wblob8": wblob8}
        in_maps.append(im)
    return in_maps, use_bias


def _gather(results):
    out = np.empty((B, N, C), np.float32)
    for core in range(NCORES):
        b, h = divmod(core, 2)
        # device layout [8 groups, 128, 2, 256] -> [2048, 256]
        o = results[core]["out"].reshape(8, 128, 2, C).transpose(0, 2, 1, 3).reshape(NQ, C)
        out[b, NQ * h : NQ * (h + 1)] = o
    return out.reshape(B, HH, WW, C)


def kernel(x, wq, bq, wk, bk, wv, bv, wo, bo):
    from concourse.bass_utils import run_bass_kernel_spmd

    in_maps, use_bias = _prep(x, wq, bq, wk, bk, wv, bv, wo, bo)
    nc = _get_compiled(use_bias)
    res = run_bass_kernel_spmd(nc, in_maps, core_ids=list(range(NCORES)))
    return _gather(res.results)


def _ensure_ntff_hook():
    """The agent image's antenv stub lacks axon_hooks; synthesize it so
    run_bass_kernel_spmd(trace=True) can NTFF-profile via libaxon_pjrt."""
    import types

    try:
        from antenv.axon_hooks import get_axon_ntff_profile_hook  # noqa: F401
        return
    except ImportError:
        pass
    import antenv
    from trn_agent_boot.trn_boot import _ntff_profile_via_ctypes

    mod = types.ModuleType("antenv.axon_hooks")
    state = {"h": _ntff_profile_via_ctypes("/opt/axon/libaxon_pjrt.so")}
    mod.get_axon_ntff_profile_hook = lambda: state["h"]
    mod.set_axon_ntff_profile_hook = lambda h: state.__setitem__("h", h)
    sys.modules["antenv.axon_hooks"] = mod
    antenv.axon_hooks = mod


def run_traced(inputs, **kw):
    """For test.py: run with NTFF profiling; returns (output, BassKernelResults)."""
    from concourse.bass_utils import run_bass_kernel_spmd

    _ensure_ntff_hook()

    in_maps, use_bias = _prep(**inputs)
    nc = _get_compiled(use_bias)
    res = run_bass_kernel_spmd(nc, in_maps, core_ids=list(range(NCORES)), trace=True, **kw)
    return _gather(res.results), res
